# revision 5
# baseline (speedup 1.0000x reference)
"""AlphaFold-style NoGatingAttention on 8 Trainium2 NeuronCores (v2).

Problem (hardcoded): B=128, Q=K=384, A=M=256, H=8, KD=VD=32, OUT=256, fp32 I/O.

Strategy: data-parallel over batch (16 per core). Per batch, on-device:
  qT = Wq^T @ q_data^T            [hc, q]   (scale folded into Wq on host)
  kT = Wk^T @ m_data^T            [hc, k]
  v  = m_data^T.T @ Wv            [k, hc]   scaled by EB=exp(bias) at evac
  logits^T[k,q] per head = kT_h^T-slices @ qT_h   (4-way row-tiled)
  E = exp(logits^T)               (ScalarE, fp16 out -- the bottleneck engine)
  E *= exp(nb)                    (DVE/GpSimd, all heads)
  waT[c,q] per head = v_h^T @ E_h  -- swapped operands, 4-way col-tiled:
      psw banks get head h at partitions (h%4)*32, i.e. the exact [hc, q]
      layout the output projection wants. No transposes, no per-MM LDWEIGHTS.
  den[c,q] = (EB 1-col bcast to 32)^T @ E_h  -- same col-tiling: denominator
      replicated across each head's 32 rows; full-bank recip -> scales tile.
  waT_n = psw * rden               (one DVE op per 4-head bank)
  out[q, o] = waT_n^T-chunks @ Wo  (+ output_b added on host)

PSUM budget (8 banks): psl ring 2x[128,2,512] = 4 banks; psA ring (1 tag,
bufs=4) = 4 banks serving psw0/psw1/den0/den1/pso*3/proj*7 per batch with
ring distances aligned to natural dependencies.
"""

import numpy as np

import concourse.bass as bass
import concourse.mybir as mybir
import concourse.tile as tile
from concourse import bacc
from concourse.bass_utils import run_bass_kernel_spmd

B, Q, KL, A_DIM, H, KD, VD, OUT = 128, 384, 384, 256, 8, 32, 32, 256
NCORES = 8
BPC = B // NCORES  # 16 batches per core
HC = H * KD  # 256
F16 = mybir.dt.float16
F32 = mybir.dt.float32

# (t, pair) ENB multiplies routed to GpSimd instead of DVE (load balance).
# GpSimd cannot touch PSUM, so all PSUM evacuations stay on DVE.
ENB_GPS = {(0, 0), (0, 1), (1, 0), (1, 1), (2, 0)}
V_EVAC_GPS = False
OUT_EVAC_GPS = False

_CACHE = {}


def _build_program():
    nc = bacc.Bacc("TRN2", target_bir_lowering=False, debug=False)

    qT_d = nc.dram_tensor("qT", [BPC, A_DIM, Q], F16, kind="ExternalInput")
    mT_d = nc.dram_tensor("mT", [BPC, A_DIM, KL], F16, kind="ExternalInput")
    eba_d = nc.dram_tensor("EB", [128, BPC, 3], F32, kind="ExternalInput")
    eba16_d = nc.dram_tensor("EB16", [128, BPC, 3], F16, kind="ExternalInput")
    enb_d = nc.dram_tensor("ENB", [128, 3, H, Q], F16, kind="ExternalInput")
    wq_d = nc.dram_tensor("Wq", [2, 128, HC], F16, kind="ExternalInput")
    wk_d = nc.dram_tensor("Wk", [2, 128, HC], F16, kind="ExternalInput")
    wv_d = nc.dram_tensor("Wv", [2, 128, HC], F16, kind="ExternalInput")
    wo_d = nc.dram_tensor("Wo", [2, 128, OUT], F16, kind="ExternalInput")
    out_d = nc.dram_tensor("out", [BPC, 3, 128, OUT], F32, kind="ExternalOutput")

    MUL = mybir.AluOpType.mult

    with tile.TileContext(nc) as tc:
        with (
            tc.tile_pool(name="const", bufs=1) as constp,
            tc.tile_pool(name="io", bufs=4) as iop,
            tc.tile_pool(name="proj", bufs=2) as projp,
            tc.tile_pool(name="epool", bufs=2) as ep,
            tc.tile_pool(name="wan", bufs=2) as wanp,
            tc.tile_pool(name="op", bufs=2) as outp,
            tc.tile_pool(name="psL", bufs=2, space="PSUM") as psL,
            tc.tile_pool(name="psA", bufs=4, space="PSUM") as psA,
        ):
            # --- constants on the gpsimd (SWDGE) queue; small weights first ---
            w_sb = {}
            for name, d in (("q", wq_d), ("k", wk_d), ("v", wv_d), ("o", wo_d)):
                w = constp.tile([128, 2, HC], F16, tag=f"w{name}")
                nc.gpsimd.dma_start(out=w, in_=d.rearrange("c p n -> p c n"))
                w_sb[name] = w
            eba = constp.tile([128, BPC, 3], F32)
            nc.gpsimd.dma_start(out=eba, in_=eba_d[:])
            eba16 = constp.tile([128, BPC, 3], F16)
            nc.gpsimd.dma_start(out=eba16, in_=eba16_d[:])
            enb_sb = constp.tile([128, 3, H, Q], F16)
            nc.gpsimd.dma_start(out=enb_sb, in_=enb_d[:])

            def dma_in(b):
                qTt = iop.tile([128, 2, Q], F16, tag="qT")
                nc.sync.dma_start(
                    out=qTt, in_=qT_d[b].rearrange("(c p) q -> p c q", p=128)
                )
                mTt = iop.tile([128, 2, KL], F16, tag="mT")
                nc.sync.dma_start(
                    out=mTt, in_=mT_d[b].rearrange("(c p) q -> p c q", p=128)
                )
                return qTt, mTt

            def proj(b, qTt, mTt):
                """q/k/v projections for batch b."""
                pq = projp.tile([128, 2, Q], F16, tag="pq")
                pk = projp.tile([128, 2, Q], F16, tag="pk")
                for dst, src, wname in ((pq, qTt, "q"), (pk, mTt, "k")):
                    for t2 in range(2):
                        ps = psA.tile([128, Q], F32, tag="A")
                        for c in range(2):
                            nc.tensor.matmul(
                                ps,
                                w_sb[wname][:, c, t2 * 128 : (t2 + 1) * 128],
                                src[:, c, :],
                                start=(c == 0),
                                stop=(c == 1),
                            )
                        nc.vector.tensor_copy(dst[:, t2, :], ps)
                v_sb = projp.tile([128, 3, HC], F16, tag="v")
                ebt = eba[:, b, :]
                for t in range(3):
                    ps = psA.tile([128, HC], F32, tag="A")
                    for c in range(2):
                        nc.tensor.matmul(
                            ps,
                            mTt[:, c, t * 128 : (t + 1) * 128],
                            w_sb["v"][:, c, :],
                            start=(c == 0),
                            stop=(c == 1),
                        )
                    eng = nc.gpsimd if V_EVAC_GPS else nc.vector
                    eng.tensor_scalar(
                        out=v_sb[:, t, :],
                        in0=ps,
                        scalar1=ebt[:, t : t + 1],
                        scalar2=None,
                        op0=MUL,
                    )
                return pq, pk, v_sb

            def qk_exp(b, pq, pk):
                """QK logits^T + exp + ENB multiply -> E_sb."""
                E_sb = ep.tile([128, 3, H, Q], F16, tag="E")
                for t in range(3):
                    for w in range(2):
                        pls = []
                        for _ in range(2):
                            psl = psL.tile([128, 2, 512], F32, tag="psl")
                            pls.append(psl)
                        for j in range(4):
                            h = w * 4 + j
                            r, ch = (h % 4) * 32, h // 4
                            nc.tensor.matmul(
                                pls[j // 2][:, j % 2, 0:Q],
                                pk[r : r + 32, ch, t * 128 : (t + 1) * 128],
                                pq[r : r + 32, ch, :],
                                start=True,
                                stop=True,
                                tile_position=(r, 0),
                            )
                        for g in range(2):
                            pair = w * 2 + g
                            nc.scalar.activation(
                                out=E_sb[:, t, pair * 2 : pair * 2 + 2, :],
                                in_=pls[g][:, :, 0:Q],
                                func=mybir.ActivationFunctionType.Exp,
                            )
                            eng = nc.gpsimd if (t, pair) in ENB_GPS else nc.vector
                            eng.tensor_tensor(
                                out=E_sb[:, t, pair * 2 : pair * 2 + 2, :],
                                in0=E_sb[:, t, pair * 2 : pair * 2 + 2, :],
                                in1=enb_sb[:, t, pair * 2 : pair * 2 + 2, :],
                                op=MUL,
                            )
                return E_sb

            def wa_out(b, v_sb, E_sb):
                """Weighted average + denominators + normalize + out proj."""
                psw0 = psA.tile([128, Q], F32, tag="A")
                psw1 = psA.tile([128, Q], F32, tag="A")
                den0 = psA.tile([128, Q], F32, tag="A")
                den1 = psA.tile([128, Q], F32, tag="A")
                psw = (psw0, psw1)
                den = (den0, den1)
                ebt16 = eba16[:, b, :]
                for t in range(3):
                    for w in range(2):
                        for j in range(4):
                            h = w * 4 + j
                            nc.tensor.matmul(
                                psw[w][32 * j : 32 * j + 32, :],
                                v_sb[:, t, h * 32 : (h + 1) * 32],
                                E_sb[:, t, h, :],
                                start=(t == 0),
                                stop=(t == 2),
                                tile_position=(0, 32 * j),
                            )
                        for j in range(4):
                            h = w * 4 + j
                            nc.tensor.matmul(
                                den[w][32 * j : 32 * j + 32, :],
                                ebt16[:, t : t + 1].broadcast_to((128, 32)),
                                E_sb[:, t, h, :],
                                start=(t == 0),
                                stop=(t == 2),
                                tile_position=(0, 32 * j),
                            )
                waT = wanp.tile([128, 2, Q], F16, tag="waT")
                rden = wanp.tile([128, 2, Q], F32, tag="rden")
                for w in range(2):
                    nc.vector.reciprocal(rden[:, w, :], den[w])
                    nc.vector.tensor_tensor(
                        out=waT[:, w, :], in0=psw[w], in1=rden[:, w, :], op=MUL
                    )
                out_sb = outp.tile([128, 3, OUT], F32, tag="out")
                for qb in range(3):
                    pso = psA.tile([128, OUT], F32, tag="A")
                    for w in range(2):
                        nc.tensor.matmul(
                            pso,
                            waT[:, w, qb * 128 : (qb + 1) * 128],
                            w_sb["o"][:, w, :],
                            start=(w == 0),
                            stop=(w == 1),
                        )
                    eng = nc.gpsimd if OUT_EVAC_GPS else nc.vector
                    eng.tensor_copy(out_sb[:, qb, :], pso)
                nc.sync.dma_start(
                    out=out_d[b].rearrange("t p n -> p t n"), in_=out_sb
                )

            # --- software-pipelined main loop ---
            qTt, mTt = dma_in(0)
            pqkv = proj(0, qTt, mTt)
            for b in range(BPC):
                if b + 1 < BPC:
                    qn, mn = dma_in(b + 1)
                pq, pk, v_sb = pqkv
                E_sb = qk_exp(b, pq, pk)
                if b + 1 < BPC:
                    pqkv = proj(b + 1, qn, mn)
                wa_out(b, v_sb, E_sb)

    nc.compile()
    return nc


def _get_program():
    if "nc" not in _CACHE:
        _CACHE["nc"] = _build_program()
    return _CACHE["nc"]


def _prep_inputs(q_data, m_data, bias, nonbatched_bias, query_w, key_w, value_w,
                 output_w):
    """Host-side layout prep -> per-core input maps."""
    scale = KD ** -0.5
    q_data = np.asarray(q_data, np.float32)
    m_data = np.asarray(m_data, np.float32)
    qT = np.ascontiguousarray(q_data.transpose(0, 2, 1)).astype(np.float16)
    mT = np.ascontiguousarray(m_data.transpose(0, 2, 1)).astype(np.float16)
    # [128, BPC, 3] per core with k = t*128 + p
    eb = np.ascontiguousarray(
        np.exp(np.asarray(bias, np.float32).reshape(B, KL))
        .reshape(NCORES, BPC, 3, 128)
        .transpose(0, 3, 1, 2)
    ).astype(np.float32)
    # [p, t, h, q] with k = t*128 + p, exp'd
    enb = np.exp(
        np.ascontiguousarray(
            np.asarray(nonbatched_bias, np.float32)
            .transpose(0, 2, 1)
            .reshape(H, 3, 128, Q)
            .transpose(2, 1, 0, 3)
        )
    ).astype(np.float16)
    wq = (np.asarray(query_w, np.float32).reshape(A_DIM, HC) * scale).reshape(
        2, 128, HC
    ).astype(np.float16)
    wk = np.asarray(key_w, np.float32).reshape(A_DIM, HC).reshape(2, 128, HC).astype(
        np.float16
    )
    wv = np.asarray(value_w, np.float32).reshape(A_DIM, HC).reshape(2, 128, HC).astype(
        np.float16
    )
    wo = np.asarray(output_w, np.float32).reshape(HC, OUT).reshape(2, 128, OUT).astype(
        np.float16
    )

    in_maps = []
    for c in range(NCORES):
        s = slice(c * BPC, (c + 1) * BPC)
        in_maps.append(
            {
                "qT": qT[s],
                "mT": mT[s],
                "EB": eb[c],
                "EB16": eb[c].astype(np.float16),
                "ENB": enb,
                "Wq": wq,
                "Wk": wk,
                "Wv": wv,
                "Wo": wo,
            }
        )
    return in_maps


def run_on_cores(in_maps, trace=False, **kwargs):
    nc = _get_program()
    return run_bass_kernel_spmd(
        nc, in_maps, core_ids=list(range(NCORES)), trace=trace, **kwargs
    )


def kernel(q_data, m_data, bias, nonbatched_bias, query_w, key_w, value_w,
           output_w, output_b):
    in_maps = _prep_inputs(
        q_data, m_data, bias, nonbatched_bias, query_w, key_w, value_w, output_w
    )
    res = run_on_cores(in_maps, trace=False)
    out = np.concatenate(
        [r["out"].reshape(BPC, Q, OUT) for r in res.results], axis=0
    ).astype(np.float32)
    out += np.asarray(output_b, np.float32)[None, None, :]
    return out


# revision 10
# speedup vs baseline: 1.0474x; 1.0474x over previous
"""AlphaFold-style NoGatingAttention on 8 Trainium2 NeuronCores (v2).

Problem (hardcoded): B=128, Q=K=384, A=M=256, H=8, KD=VD=32, OUT=256, fp32 I/O.

Strategy: data-parallel over batch (16 per core). Per batch, on-device:
  qT = Wq^T @ q_data^T            [hc, q]   (scale folded into Wq on host)
  kT = Wk^T @ m_data^T            [hc, k]
  v  = m_data^T.T @ Wv            [k, hc]   scaled by EB=exp(bias) at evac
  logits^T[k,q] per head = kT_h^T-slices @ qT_h   (4-way row-tiled)
  E = exp(logits^T)               (ScalarE, fp16 out -- the bottleneck engine)
  E *= exp(nb)                    (DVE/GpSimd, all heads)
  waT[c,q] per head = v_h^T @ E_h  -- swapped operands, 4-way col-tiled:
      psw banks get head h at partitions (h%4)*32, i.e. the exact [hc, q]
      layout the output projection wants. No transposes, no per-MM LDWEIGHTS.
  den[c,q] = (EB 1-col bcast to 32)^T @ E_h  -- same col-tiling: denominator
      replicated across each head's 32 rows; full-bank recip -> scales tile.
  waT_n = psw * rden               (one DVE op per 4-head bank)
  out[q, o] = waT_n^T-chunks @ Wo  (+ output_b added on host)

PSUM budget (8 banks): psl ring 2x[128,2,512] = 4 banks; psA ring (1 tag,
bufs=4) = 4 banks serving psw0/psw1/den0/den1/pso*3/proj*7 per batch with
ring distances aligned to natural dependencies.
"""

import numpy as np

import concourse.bass as bass
import concourse.mybir as mybir
import concourse.tile as tile
from concourse import bacc
from concourse.bass_utils import run_bass_kernel_spmd

B, Q, KL, A_DIM, H, KD, VD, OUT = 128, 384, 384, 256, 8, 32, 32, 256
NCORES = 8
BPC = B // NCORES  # 16 batches per core
HC = H * KD  # 256
F16 = mybir.dt.float16
F32 = mybir.dt.float32

# (t, pair) ENB multiplies routed to GpSimd instead of DVE (load balance).
# GpSimd cannot touch PSUM, so all PSUM evacuations stay on DVE.
ENB_GPS = {(0, 0), (0, 1), (1, 0), (1, 1)}
V_EVAC_GPS = False
OUT_EVAC_GPS = False

_CACHE = {}


def _build_program():
    nc = bacc.Bacc("TRN2", target_bir_lowering=False, debug=False)

    qT_d = nc.dram_tensor("qT", [BPC, A_DIM, Q], F16, kind="ExternalInput")
    mT_d = nc.dram_tensor("mT", [BPC, A_DIM, KL], F16, kind="ExternalInput")
    eba_d = nc.dram_tensor("EB", [128, BPC, 3], F32, kind="ExternalInput")
    eba16_d = nc.dram_tensor("EB16", [128, BPC, 3], F16, kind="ExternalInput")
    enb_d = nc.dram_tensor("ENB", [128, 3, H, Q], F16, kind="ExternalInput")
    wq_d = nc.dram_tensor("Wq", [2, 128, HC], F16, kind="ExternalInput")
    wk_d = nc.dram_tensor("Wk", [2, 128, HC], F16, kind="ExternalInput")
    wv_d = nc.dram_tensor("Wv", [2, 128, HC], F16, kind="ExternalInput")
    wo_d = nc.dram_tensor("Wo", [2, 128, OUT], F16, kind="ExternalInput")
    out_d = nc.dram_tensor("out", [BPC, 3, 128, OUT], F32, kind="ExternalOutput")

    MUL = mybir.AluOpType.mult

    with tile.TileContext(nc) as tc:
        with (
            tc.tile_pool(name="const", bufs=1) as constp,
            tc.tile_pool(name="io", bufs=4) as iop,
            tc.tile_pool(name="proj", bufs=2) as projp,
            tc.tile_pool(name="epool", bufs=2) as ep,
            tc.tile_pool(name="wan", bufs=2) as wanp,
            tc.tile_pool(name="op", bufs=2) as outp,
            tc.tile_pool(name="psL", bufs=2, space="PSUM") as psL,
            tc.tile_pool(name="psA", bufs=4, space="PSUM") as psA,
        ):
            # --- constants on the gpsimd (SWDGE) queue; small weights first ---
            w_sb = {}
            for name, d in (("q", wq_d), ("k", wk_d), ("v", wv_d), ("o", wo_d)):
                w = constp.tile([128, 2, HC], F16, tag=f"w{name}")
                nc.gpsimd.dma_start(out=w, in_=d.rearrange("c p n -> p c n"))
                w_sb[name] = w
            eba = constp.tile([128, BPC, 3], F32)
            nc.gpsimd.dma_start(out=eba, in_=eba_d[:])
            eba16 = constp.tile([128, BPC, 3], F16)
            nc.gpsimd.dma_start(out=eba16, in_=eba16_d[:])
            enb_sb = constp.tile([128, 3, H, Q], F16)
            nc.gpsimd.dma_start(out=enb_sb, in_=enb_d[:])

            def dma_in(b):
                qTt = iop.tile([128, 2, Q], F16, tag="qT")
                nc.sync.dma_start(
                    out=qTt, in_=qT_d[b].rearrange("(c p) q -> p c q", p=128)
                )
                mTt = iop.tile([128, 2, KL], F16, tag="mT")
                nc.sync.dma_start(
                    out=mTt, in_=mT_d[b].rearrange("(c p) q -> p c q", p=128)
                )
                return qTt, mTt

            def proj(b, qTt, mTt):
                """q/k/v projections for batch b."""
                pq = projp.tile([128, 2, Q], F16, tag="pq")
                pk = projp.tile([128, 2, Q], F16, tag="pk")
                for dst, src, wname in ((pq, qTt, "q"), (pk, mTt, "k")):
                    for t2 in range(2):
                        ps = psA.tile([128, Q], F32, tag="A")
                        for c in range(2):
                            nc.tensor.matmul(
                                ps,
                                w_sb[wname][:, c, t2 * 128 : (t2 + 1) * 128],
                                src[:, c, :],
                                start=(c == 0),
                                stop=(c == 1),
                            )
                        nc.vector.tensor_copy(dst[:, t2, :], ps)
                v_sb = projp.tile([128, 3, HC], F16, tag="v")
                ebt = eba[:, b, :]
                for t in range(3):
                    ps = psA.tile([128, HC], F32, tag="A")
                    for c in range(2):
                        nc.tensor.matmul(
                            ps,
                            mTt[:, c, t * 128 : (t + 1) * 128],
                            w_sb["v"][:, c, :],
                            start=(c == 0),
                            stop=(c == 1),
                        )
                    eng = nc.gpsimd if V_EVAC_GPS else nc.vector
                    eng.tensor_scalar(
                        out=v_sb[:, t, :],
                        in0=ps,
                        scalar1=ebt[:, t : t + 1],
                        scalar2=None,
                        op0=MUL,
                    )
                return pq, pk, v_sb

            def qk_exp(b, pq, pk):
                """QK logits^T + exp + ENB multiply -> E_sb."""
                E_sb = ep.tile([128, 3, H, Q], F16, tag="E")
                for t in range(3):
                    for w in range(2):
                        pls = []
                        for _ in range(2):
                            psl = psL.tile([128, 2, 512], F32, tag="psl")
                            pls.append(psl)
                        for j in range(4):
                            h = w * 4 + j
                            r, ch = (h % 4) * 32, h // 4
                            nc.tensor.matmul(
                                pls[j // 2][:, j % 2, 0:Q],
                                pk[r : r + 32, ch, t * 128 : (t + 1) * 128],
                                pq[r : r + 32, ch, :],
                                start=True,
                                stop=True,
                                tile_position=(r, 0),
                            )
                        for g in range(2):
                            pair = w * 2 + g
                            nc.scalar.activation(
                                out=E_sb[:, t, pair * 2 : pair * 2 + 2, :],
                                in_=pls[g][:, :, 0:Q],
                                func=mybir.ActivationFunctionType.Exp,
                            )
                            eng = nc.gpsimd if (t, pair) in ENB_GPS else nc.vector
                            eng.tensor_tensor(
                                out=E_sb[:, t, pair * 2 : pair * 2 + 2, :],
                                in0=E_sb[:, t, pair * 2 : pair * 2 + 2, :],
                                in1=enb_sb[:, t, pair * 2 : pair * 2 + 2, :],
                                op=MUL,
                            )
                return E_sb

            def wa_out(b, v_sb, E_sb):
                """Weighted average + denominators + normalize + out proj."""
                psw0 = psA.tile([128, Q], F32, tag="A")
                psw1 = psA.tile([128, Q], F32, tag="A")
                den0 = psA.tile([128, Q], F32, tag="A")
                den1 = psA.tile([128, Q], F32, tag="A")
                psw = (psw0, psw1)
                den = (den0, den1)
                ebt16 = eba16[:, b, :]
                for t in range(3):
                    for w in range(2):
                        for j in range(4):
                            h = w * 4 + j
                            nc.tensor.matmul(
                                psw[w][32 * j : 32 * j + 32, :],
                                v_sb[:, t, h * 32 : (h + 1) * 32],
                                E_sb[:, t, h, :],
                                start=(t == 0),
                                stop=(t == 2),
                                tile_position=(0, 32 * j),
                            )
                        for j in range(4):
                            h = w * 4 + j
                            nc.tensor.matmul(
                                den[w][32 * j : 32 * j + 32, :],
                                ebt16[:, t : t + 1].broadcast_to((128, 32)),
                                E_sb[:, t, h, :],
                                start=(t == 0),
                                stop=(t == 2),
                                tile_position=(0, 32 * j),
                            )
                waT = wanp.tile([128, 2, Q], F16, tag="waT")
                rden = wanp.tile([128, 2, Q], F32, tag="rden")
                for w in range(2):
                    nc.vector.reciprocal_approx_fast(rden[:, w, :], den[w])
                    nc.vector.tensor_tensor(
                        out=waT[:, w, :], in0=psw[w], in1=rden[:, w, :], op=MUL
                    )
                out_sb = outp.tile([128, 3, OUT], F32, tag="out")
                for qb in range(3):
                    pso = psA.tile([128, OUT], F32, tag="A")
                    for w in range(2):
                        nc.tensor.matmul(
                            pso,
                            waT[:, w, qb * 128 : (qb + 1) * 128],
                            w_sb["o"][:, w, :],
                            start=(w == 0),
                            stop=(w == 1),
                        )
                    eng = nc.scalar if qb == 2 else nc.vector
                    eng.copy(out_sb[:, qb, :], pso) if qb == 2 else \
                        nc.vector.tensor_copy(out_sb[:, qb, :], pso)
                nc.sync.dma_start(
                    out=out_d[b].rearrange("t p n -> p t n"), in_=out_sb
                )

            # --- software-pipelined main loop ---
            qTt, mTt = dma_in(0)
            pqkv = proj(0, qTt, mTt)
            for b in range(BPC):
                if b + 1 < BPC:
                    qn, mn = dma_in(b + 1)
                pq, pk, v_sb = pqkv
                E_sb = qk_exp(b, pq, pk)
                if b + 1 < BPC:
                    pqkv = proj(b + 1, qn, mn)
                wa_out(b, v_sb, E_sb)

    nc.compile()
    return nc


def _get_program():
    if "nc" not in _CACHE:
        _CACHE["nc"] = _build_program()
    return _CACHE["nc"]


def _prep_inputs(q_data, m_data, bias, nonbatched_bias, query_w, key_w, value_w,
                 output_w):
    """Host-side layout prep -> per-core input maps."""
    scale = KD ** -0.5
    q_data = np.asarray(q_data, np.float32)
    m_data = np.asarray(m_data, np.float32)
    qT = np.ascontiguousarray(q_data.transpose(0, 2, 1)).astype(np.float16)
    mT = np.ascontiguousarray(m_data.transpose(0, 2, 1)).astype(np.float16)
    # [128, BPC, 3] per core with k = t*128 + p
    eb = np.ascontiguousarray(
        np.exp(np.asarray(bias, np.float32).reshape(B, KL))
        .reshape(NCORES, BPC, 3, 128)
        .transpose(0, 3, 1, 2)
    ).astype(np.float32)
    # [p, t, h, q] with k = t*128 + p, exp'd
    enb = np.exp(
        np.ascontiguousarray(
            np.asarray(nonbatched_bias, np.float32)
            .transpose(0, 2, 1)
            .reshape(H, 3, 128, Q)
            .transpose(2, 1, 0, 3)
        )
    ).astype(np.float16)
    wq = (np.asarray(query_w, np.float32).reshape(A_DIM, HC) * scale).reshape(
        2, 128, HC
    ).astype(np.float16)
    wk = np.asarray(key_w, np.float32).reshape(A_DIM, HC).reshape(2, 128, HC).astype(
        np.float16
    )
    wv = np.asarray(value_w, np.float32).reshape(A_DIM, HC).reshape(2, 128, HC).astype(
        np.float16
    )
    wo = np.asarray(output_w, np.float32).reshape(HC, OUT).reshape(2, 128, OUT).astype(
        np.float16
    )

    in_maps = []
    for c in range(NCORES):
        s = slice(c * BPC, (c + 1) * BPC)
        in_maps.append(
            {
                "qT": qT[s],
                "mT": mT[s],
                "EB": eb[c],
                "EB16": eb[c].astype(np.float16),
                "ENB": enb,
                "Wq": wq,
                "Wk": wk,
                "Wv": wv,
                "Wo": wo,
            }
        )
    return in_maps


def run_on_cores(in_maps, trace=False, **kwargs):
    nc = _get_program()
    return run_bass_kernel_spmd(
        nc, in_maps, core_ids=list(range(NCORES)), trace=trace, **kwargs
    )


def kernel(q_data, m_data, bias, nonbatched_bias, query_w, key_w, value_w,
           output_w, output_b):
    in_maps = _prep_inputs(
        q_data, m_data, bias, nonbatched_bias, query_w, key_w, value_w, output_w
    )
    res = run_on_cores(in_maps, trace=False)
    out = np.concatenate(
        [r["out"].reshape(BPC, Q, OUT) for r in res.results], axis=0
    ).astype(np.float32)
    out += np.asarray(output_b, np.float32)[None, None, :]
    return out


# revision 11
# speedup vs baseline: 1.1220x; 1.0712x over previous
"""AlphaFold-style NoGatingAttention on 8 Trainium2 NeuronCores (v3).

Problem (hardcoded): B=128, Q=K=384, A=M=256, H=8, KD=VD=32, OUT=256, fp32 I/O.

Data-parallel over batch (16 per core). Per batch:
  qT = Wq^T @ q_data^T            [hc, q]   (1/sqrt(kd) folded into Wq)
  kT = Wk^T @ m_data^T            [hc, k]
  v  = m_data^T.T @ Wv            [k, hc]
  logits^T[k,q] per head = kT_h^T-slices @ qT_h   (4-way row-tiled)
  E = exp(logits^T + (bias_b[k] - C))   ACT per-partition bias operand; the
      global shift C keeps E*ENB inside f16 range (softmax-invariant)
  E2 = E * exp(nb)                (DVE/GpSimd, separate tile - no in-place)
  waT[c,q] per head = v_h^T @ E2_h   swapped operands, 4-way col-tiled:
      psw banks hold head h at partitions (h%4)*32 -> exact outproj layout.
  den = (ones 1-col bcast to 32)^T @ E2_h  same col-tiling: denominator
      replicated per 32-row group; full-bank reciprocal_approx_fast -> scales.
  waT_n = psw * rden; out[q,o] = waT_n^T-chunks @ Wo (+output_b on host)

Pipelining: the PE instruction stream is FIFO, so batch b-1's WA/den matmuls
are emitted *between* batch b's QK waves (they become ready exactly when
window b starts). proj(b+1) is emitted after the t=1 slice so its PSUM tiles
(from the psL ring) recycle mid-window. PSUM: psL ring 2x[128,2,512] (4
banks) serves 12 QK + 2 proj-qk + 3 proj-v tiles/batch; psA ring (bufs=4)
serves psw0/psw1/den0/den1/pso*3.
"""

import numpy as np

import concourse.bass as bass
import concourse.mybir as mybir
import concourse.tile as tile
from concourse import bacc
from concourse.bass_utils import run_bass_kernel_spmd

B, Q, KL, A_DIM, H, KD, VD, OUT = 128, 384, 384, 256, 8, 32, 32, 256
NCORES = 8
BPC = B // NCORES  # 16 batches per core
HC = H * KD  # 256
F16 = mybir.dt.float16
F32 = mybir.dt.float32
BIAS_SHIFT = 4.0  # global logit shift (softmax-invariant, keeps f16 in range)

# (t, pair) ENB multiplies routed to GpSimd instead of DVE (load balance).
ENB_GPS = {(0, 0), (0, 1), (1, 0), (1, 1), (2, 0)}

_CACHE = {}


def _build_program():
    nc = bacc.Bacc("TRN2", target_bir_lowering=False, debug=False)

    qT_d = nc.dram_tensor("qT", [BPC, A_DIM, Q], F16, kind="ExternalInput")
    mT_d = nc.dram_tensor("mT", [BPC, A_DIM, KL], F16, kind="ExternalInput")
    bias_d = nc.dram_tensor("BIAS", [128, BPC, 3], F32, kind="ExternalInput")
    enb_d = nc.dram_tensor("ENB", [128, 3, H, Q], F16, kind="ExternalInput")
    wq_d = nc.dram_tensor("Wq", [2, 128, HC], F16, kind="ExternalInput")
    wk_d = nc.dram_tensor("Wk", [2, 128, HC], F16, kind="ExternalInput")
    wv_d = nc.dram_tensor("Wv", [2, 128, HC], F16, kind="ExternalInput")
    wo_d = nc.dram_tensor("Wo", [2, 128, OUT], F16, kind="ExternalInput")
    out_d = nc.dram_tensor("out", [BPC, 3, 128, OUT], F32, kind="ExternalOutput")

    MUL = mybir.AluOpType.mult

    with tile.TileContext(nc) as tc:
        with (
            tc.tile_pool(name="const", bufs=1) as constp,
            tc.tile_pool(name="io", bufs=4) as iop,
            tc.tile_pool(name="proj", bufs=2) as projp,
            tc.tile_pool(name="epool", bufs=2) as ep,
            tc.tile_pool(name="wan", bufs=2) as wanp,
            tc.tile_pool(name="op", bufs=2) as outp,
            tc.tile_pool(name="psL", bufs=2, space="PSUM") as psL,
            tc.tile_pool(name="psA", bufs=4, space="PSUM") as psA,
        ):
            # --- constants on the gpsimd (SWDGE) queue; small weights first ---
            w_sb = {}
            for name, d in (("q", wq_d), ("k", wk_d), ("v", wv_d), ("o", wo_d)):
                w = constp.tile([128, 2, HC], F16, tag=f"w{name}")
                nc.gpsimd.dma_start(out=w, in_=d.rearrange("c p n -> p c n"))
                w_sb[name] = w
            bias_sb = constp.tile([128, BPC, 3], F32)
            nc.gpsimd.dma_start(out=bias_sb, in_=bias_d[:])
            enb_sb = constp.tile([128, 3, H, Q], F16)
            nc.gpsimd.dma_start(out=enb_sb, in_=enb_d[:])
            ones16 = constp.tile([128, 1], F16)
            nc.vector.memset(ones16, 1.0)

            def dma_in(b):
                qTt = iop.tile([128, 2, Q], F16, tag="qT")
                nc.sync.dma_start(
                    out=qTt, in_=qT_d[b].rearrange("(c p) q -> p c q", p=128)
                )
                mTt = iop.tile([128, 2, KL], F16, tag="mT")
                nc.sync.dma_start(
                    out=mTt, in_=mT_d[b].rearrange("(c p) q -> p c q", p=128)
                )
                return qTt, mTt

            def proj(b, qTt, mTt):
                """q/k/v projections for batch b. q/k psum from the psL ring."""
                pq = projp.tile([128, 2, Q], F16, tag="pq")
                pk = projp.tile([128, 2, Q], F16, tag="pk")
                for dst, src, wname in ((pq, qTt, "q"), (pk, mTt, "k")):
                    ps2 = psL.tile([128, 2, 512], F32, tag="psl")
                    for t2 in range(2):
                        for c in range(2):
                            nc.tensor.matmul(
                                ps2[:, t2, 0:Q],
                                w_sb[wname][:, c, t2 * 128 : (t2 + 1) * 128],
                                src[:, c, :],
                                start=(c == 0),
                                stop=(c == 1),
                            )
                    nc.vector.tensor_copy(dst, ps2[:, :, 0:Q])
                v_sb = projp.tile([128, 3, HC], F16, tag="v")
                for t in range(3):
                    ps = psA.tile([128, HC], F32, tag="A")
                    for c in range(2):
                        nc.tensor.matmul(
                            ps,
                            mTt[:, c, t * 128 : (t + 1) * 128],
                            w_sb["v"][:, c, :],
                            start=(c == 0),
                            stop=(c == 1),
                        )
                    nc.vector.tensor_copy(v_sb[:, t, :], ps)
                return pq, pk, v_sb

            def qk_slice(b, t, pq, pk, E_sb, E2_sb):
                """One k-tile of QK + exp(+bias) + ENB for batch b."""
                for w in range(2):
                    pls = []
                    for _ in range(2):
                        psl = psL.tile([128, 2, 512], F32, tag="psl")
                        pls.append(psl)
                    for j in range(4):
                        h = w * 4 + j
                        r, ch = (h % 4) * 32, h // 4
                        nc.tensor.matmul(
                            pls[j // 2][:, j % 2, 0:Q],
                            pk[r : r + 32, ch, t * 128 : (t + 1) * 128],
                            pq[r : r + 32, ch, :],
                            start=True,
                            stop=True,
                            tile_position=(r, 0),
                        )
                    for g in range(2):
                        pair = w * 2 + g
                        nc.scalar.activation(
                            out=E_sb[:, t, pair * 2 : pair * 2 + 2, :],
                            in_=pls[g][:, :, 0:Q],
                            func=mybir.ActivationFunctionType.Exp,
                            bias=bias_sb[:, b, t : t + 1],
                        )
                        eng = nc.gpsimd if (t, pair) in ENB_GPS else nc.vector
                        eng.tensor_tensor(
                            out=E2_sb[:, t, pair * 2 : pair * 2 + 2, :],
                            in0=E_sb[:, t, pair * 2 : pair * 2 + 2, :],
                            in1=enb_sb[:, t, pair * 2 : pair * 2 + 2, :],
                            op=MUL,
                        )

            def wa_slice(t, st):
                """One k-tile of WA + den matmuls for a previous batch."""
                v_sb, E2_sb = st["v"], st["E2"]
                psw, den = st["psw"], st["den"]
                for w in range(2):
                    for j in range(4):
                        h = w * 4 + j
                        nc.tensor.matmul(
                            psw[w][32 * j : 32 * j + 32, :],
                            v_sb[:, t, h * 32 : (h + 1) * 32],
                            E2_sb[:, t, h, :],
                            start=(t == 0),
                            stop=(t == 2),
                            tile_position=(0, 32 * j),
                        )
                    for j in range(4):
                        h = w * 4 + j
                        nc.tensor.matmul(
                            den[w][32 * j : 32 * j + 32, :],
                            ones16.broadcast_to((128, 32)),
                            E2_sb[:, t, h, :],
                            start=(t == 0),
                            stop=(t == 2),
                            tile_position=(0, 32 * j),
                        )

            def finish(b, st):
                """Normalize + output projection + out-DMA for batch b."""
                psw, den = st["psw"], st["den"]
                waT = wanp.tile([128, 2, Q], F16, tag="waT")
                rden = wanp.tile([128, 2, Q], F32, tag="rden")
                for w in range(2):
                    nc.vector.reciprocal_approx_fast(rden[:, w, :], den[w])
                    nc.vector.tensor_tensor(
                        out=waT[:, w, :], in0=psw[w], in1=rden[:, w, :], op=MUL
                    )
                out_sb = outp.tile([128, 3, OUT], F32, tag="out")
                for qb in range(3):
                    pso = psA.tile([128, OUT], F32, tag="A")
                    for w in range(2):
                        nc.tensor.matmul(
                            pso,
                            waT[:, w, qb * 128 : (qb + 1) * 128],
                            w_sb["o"][:, w, :],
                            start=(w == 0),
                            stop=(w == 1),
                        )
                    nc.vector.tensor_copy(out_sb[:, qb, :], pso)
                nc.sync.dma_start(
                    out=out_d[b].rearrange("t p n -> p t n"), in_=out_sb
                )

            # --- software-pipelined main loop ---
            qTt, mTt = dma_in(0)
            pq, pk, v_sb = proj(0, qTt, mTt)
            prev = None  # state of batch b-1 awaiting WA/finish
            for b in range(BPC):
                if b + 1 < BPC:
                    qn, mn = dma_in(b + 1)
                E_sb = ep.tile([128, 3, H, Q], F16, tag="E")
                E2_sb = ep.tile([128, 3, H, Q], F16, tag="E2")
                if prev is not None:
                    psw0 = psA.tile([128, Q], F32, tag="A")
                    psw1 = psA.tile([128, Q], F32, tag="A")
                    den0 = psA.tile([128, Q], F32, tag="A")
                    den1 = psA.tile([128, Q], F32, tag="A")
                    prev["psw"] = (psw0, psw1)
                    prev["den"] = (den0, den1)
                for t in range(3):
                    qk_slice(b, t, pq, pk, E_sb, E2_sb)
                    if prev is not None:
                        wa_slice(t, prev)
                    if t == 1 and b + 1 < BPC:
                        nq, nk, nv = proj(b + 1, qn, mn)
                if prev is not None:
                    finish(b - 1, prev)
                prev = {"v": v_sb, "E2": E2_sb}
                if b + 1 < BPC:
                    pq, pk, v_sb = nq, nk, nv

            # drain the last batch
            psw0 = psA.tile([128, Q], F32, tag="A")
            psw1 = psA.tile([128, Q], F32, tag="A")
            den0 = psA.tile([128, Q], F32, tag="A")
            den1 = psA.tile([128, Q], F32, tag="A")
            prev["psw"] = (psw0, psw1)
            prev["den"] = (den0, den1)
            for t in range(3):
                wa_slice(t, prev)
            finish(BPC - 1, prev)

    nc.compile()
    return nc


def _get_program():
    if "nc" not in _CACHE:
        _CACHE["nc"] = _build_program()
    return _CACHE["nc"]


def _prep_inputs(q_data, m_data, bias, nonbatched_bias, query_w, key_w, value_w,
                 output_w):
    """Host-side layout prep -> per-core input maps."""
    scale = KD ** -0.5
    q_data = np.asarray(q_data, np.float32)
    m_data = np.asarray(m_data, np.float32)
    qT = np.ascontiguousarray(q_data.transpose(0, 2, 1)).astype(np.float16)
    mT = np.ascontiguousarray(m_data.transpose(0, 2, 1)).astype(np.float16)
    # raw bias, shifted: [128, BPC, 3] per core with k = t*128 + p
    bias_k = np.ascontiguousarray(
        (np.asarray(bias, np.float32).reshape(B, KL) - BIAS_SHIFT)
        .reshape(NCORES, BPC, 3, 128)
        .transpose(0, 3, 1, 2)
    ).astype(np.float32)
    # [p, t, h, q] with k = t*128 + p, exp'd
    enb = np.exp(
        np.ascontiguousarray(
            np.asarray(nonbatched_bias, np.float32)
            .transpose(0, 2, 1)
            .reshape(H, 3, 128, Q)
            .transpose(2, 1, 0, 3)
        )
    ).astype(np.float16)
    wq = (np.asarray(query_w, np.float32).reshape(A_DIM, HC) * scale).reshape(
        2, 128, HC
    ).astype(np.float16)
    wk = np.asarray(key_w, np.float32).reshape(A_DIM, HC).reshape(2, 128, HC).astype(
        np.float16
    )
    wv = np.asarray(value_w, np.float32).reshape(A_DIM, HC).reshape(2, 128, HC).astype(
        np.float16
    )
    wo = np.asarray(output_w, np.float32).reshape(HC, OUT).reshape(2, 128, OUT).astype(
        np.float16
    )

    in_maps = []
    for c in range(NCORES):
        s = slice(c * BPC, (c + 1) * BPC)
        in_maps.append(
            {
                "qT": qT[s],
                "mT": mT[s],
                "BIAS": bias_k[c],
                "ENB": enb,
                "Wq": wq,
                "Wk": wk,
                "Wv": wv,
                "Wo": wo,
            }
        )
    return in_maps


def run_on_cores(in_maps, trace=False, **kwargs):
    nc = _get_program()
    return run_bass_kernel_spmd(
        nc, in_maps, core_ids=list(range(NCORES)), trace=trace, **kwargs
    )


def kernel(q_data, m_data, bias, nonbatched_bias, query_w, key_w, value_w,
           output_w, output_b):
    in_maps = _prep_inputs(
        q_data, m_data, bias, nonbatched_bias, query_w, key_w, value_w, output_w
    )
    res = run_on_cores(in_maps, trace=False)
    out = np.concatenate(
        [r["out"].reshape(BPC, Q, OUT) for r in res.results], axis=0
    ).astype(np.float32)
    out += np.asarray(output_b, np.float32)[None, None, :]
    return out
